# revision 64
# baseline (speedup 1.0000x reference)
"""Distributed Trainium2 kernel for nn_Attention_14697378086932.

Head-sharded (tensor-parallel) multi-head attention over 8 NeuronCores:
each core computes 2 of the 16 heads end-to-end.

Per core c:
  - QKV projections:  Q^T = Wq_c^T @ X^T  (f32r matmuls, contraction on
    hidden dim), giving Q^T/K^T/V^T in [128 local channels, 4096 tokens]
    layout (d-on-partitions), which is exactly the layout the scores
    matmul wants.  Token chunks run in weight-stationary pairs so every
    second matmul skips its LDWEIGHTS.
  - rotary: only global channels 0..63 are rotated (reference quirk), i.e.
    local channels 0..63 of core 0.  All cores run the same graph; cores
    1..7 receive cos=1/sin=0 so their "rotation" is the identity.
    rotate_half is a fixed permutation matrix applied on the PE.
  - attention (per batch, per local head, flash-style over 128-token key
    chunks): S^T = K Q^T (f32r), P^T = exp(S^T) on the scalar engine
    (no max subtraction -- logits are bounded, f32/bf16 exp is safe),
    O^T = [V | 1]^T @ P^T (bf16) which yields the softmax denominator as
    a free 65th row.  Normalize with a reciprocal multiply.
  - output projection: partial = O_loc @ Wo_c (bf16), DMA'd out per
    128-token chunk, interleaved into later attention blocks.
Throughput notes: ALL matmuls are zero-padded to a full 128-row/column
array footprint -- the PE clock gate throttles half-array work to
1.2 GHz.  The S/exp/PV software pipeline carries across block
boundaries, and scratch "keep-warm" matmuls bridge the remaining PE
idle gaps.  Host sums the 8 partial outputs and adds bo.
"""
import sys
import types

sys.path.insert(0, "/opt/trn_rl_repo")

import numpy as np
import ml_dtypes

import concourse.bass as bass
import concourse.mybir as mybir
from concourse import bacc
from concourse.bass import ts, ds
from concourse.tile import TileContext
from concourse.masks import make_identity
from concourse.bass_utils import run_bass_kernel_spmd

F32 = mybir.dt.float32
F32R = mybir.dt.float32r
BF16 = mybir.dt.bfloat16

P = 128          # partitions / local channels per core
HID = 1024       # hidden
NT = 4096        # total tokens (batch 2 x 2048)
NB = 2048        # tokens per batch
HD = 64          # head dim
N_CORES = 8

_NC_CACHE = None


def build_nc():
    nc = bacc.Bacc("TRN2")

    xt = nc.declare_dram_parameter("xt", [HID, NT], F32R, isOutput=False)
    wq = nc.declare_dram_parameter("wq", [HID, P], F32R, isOutput=False)
    wk = nc.declare_dram_parameter("wk", [HID, P], F32R, isOutput=False)
    wv = nc.declare_dram_parameter("wv", [HID, P], F32R, isOutput=False)
    wo = nc.declare_dram_parameter("wo", [P, HID], BF16, isOutput=False)
    bia = nc.declare_dram_parameter("bias", [P, 3], F32, isOutput=False)
    cos = nc.declare_dram_parameter("cos", [HD, NT], BF16, isOutput=False)
    sin = nc.declare_dram_parameter("sin", [HD, NT], BF16, isOutput=False)
    rmat = nc.declare_dram_parameter("rmat", [P, P], F32R, isOutput=False)
    out = nc.declare_dram_parameter("out", [NT, HID], F32, isOutput=True)

    xt_r = xt[:].rearrange("(o p) n -> p o n", p=P)      # [128, 8, 4096]
    wq_r = wq[:].rearrange("(o p) m -> p o m", p=P)      # [128, 8, 128]
    wk_r = wk[:].rearrange("(o p) m -> p o m", p=P)
    wv_r = wv[:].rearrange("(o p) m -> p o m", p=P)

    with TileContext(nc) as tc:
        with tc.tile_pool(name="consts", bufs=1) as consts, \
             tc.tile_pool(name="big", bufs=1) as big:
            wqs = consts.tile([P, 8, P], F32R)
            wks = consts.tile([P, 8, P], F32R)
            wvs = consts.tile([P, 8, P], F32R)
            nc.sync.dma_start(wqs, wq_r)
            nc.sync.dma_start(wks, wk_r)
            nc.sync.dma_start(wvs, wv_r)
            wos = consts.tile([P, HID], BF16)
            nc.sync.dma_start(wos, wo[:])
            bias_t = consts.tile([P, 3], F32)
            nc.sync.dma_start(bias_t, bia[:])
            rmat_t = consts.tile([P, P], F32R)
            nc.sync.dma_start(rmat_t, rmat[:])
            ident = consts.tile([P, P], F32)
            make_identity(nc, ident)
            warm_exp = consts.tile([P, 1], F32)
            nc.scalar.activation(warm_exp, bias_t[:, 0:1],
                                 mybir.ActivationFunctionType.Exp)

            Qt = big.tile([P, NT], F32R)     # Q^T (local channels x tokens)
            Kt = big.tile([P, NT], F32R)
            # normalized attention out^T, one tile per (batch, nq-block)
            # pair so Tile's per-tile dependency tracking doesn't chain an
            # output-projection read onto an unrelated block's normalize
            OtT = []
            for k in range(4):
                ot_k = big.tile([P, 1024], BF16, name=f"Ot{k}")
                OtT.append(ot_k)
            # Zero-padded per-head copies of K^T: head A in rows 0..63 with
            # zeros below, head B in rows 64..127 with zeros above.  Used as
            # the S^T stationary operand with a full-128 contraction (the
            # zero rows null out the other head's Q rows), which keeps the
            # PE array fully active -- half-array matmuls let the clock
            # gate throttle the PE to 1.2 GHz.
            KzA = big.tile([P, NT], F32R)
            KzB = big.tile([P, NT], F32R)
            nc.vector.memset(KzA[HD:P, :].bitcast(F32), 0.0)
            nc.vector.memset(KzB[0:HD, :].bitcast(F32), 0.0)
            # V in natural [token, channel] layout, per head, per 128-token
            # key chunk: [128 tok, 32 chunks, 64 V | 1 | 63 zeros].  Padded
            # to a full 128 stationary columns so the PV matmul drives the
            # whole PE array (keeps the clock gate at 8/8) and qualifies
            # for fast weight load; the zero columns land in PSUM rows
            # 65..127, which the normalize simply never reads.
            VaugA = big.tile([P, 32, P], BF16)
            VaugB = big.tile([P, 32, P], BF16)
            nc.vector.memset(VaugA, 0.0)
            nc.vector.memset(VaugB, 0.0)
            nc.vector.memset(VaugA[:, :, 64:65], 1.0)
            nc.vector.memset(VaugB[:, :, 64:65], 1.0)

            # ---------------- Phase A: QKV projections + rope + V transpose
            # Token chunks are processed in pairs with the hidden-chunk (o)
            # loop outside the pair: consecutive matmuls then share their
            # stationary operand and the second skips its LDWEIGHTS.  Six
            # accumulators (3 projections x 2 token chunks) + rope + trans
            # use all 8 PSUM banks.
            with tc.tile_pool(name="xtp", bufs=3) as xtp, \
                 tc.tile_pool(name="ropet", bufs=2) as ropet, \
                 tc.tile_pool(name="trig", bufs=1) as trig, \
                 tc.tile_pool(name="vtp", bufs=1) as vtp, \
                 tc.tile_pool(name="psA", bufs=1, space="PSUM") as psA, \
                 tc.tile_pool(name="psRT", bufs=2, space="PSUM") as psRT:
                Vt = vtp.tile([P, NT], F32)   # V^T, pre-transpose
                cos_t = trig.tile([HD, NT], BF16)
                sin_t = trig.tile([HD, NT], BF16)
                nc.sync.dma_start(cos_t, cos[:])
                nc.sync.dma_start(sin_t, sin[:])
                for g in range(4):    # pairs of 512-token chunks
                    xtts = []
                    for u in range(2):
                        xtt = xtp.tile([P, 8, 512], F32R, tag="xt")
                        nc.sync.dma_start(
                            xtt, xt_r[:, :, ts(2 * g + u, 512)])
                        xtts.append(xtt)
                    for wt, bidx, dst in ((wqs, 0, Qt), (wks, 1, Kt),
                                          (wvs, 2, Vt)):
                        pss = [psA.tile([P, 512], F32, tag=f"ps{bidx}{u}",
                                        name=f"ps{bidx}{u}")
                               for u in range(2)]
                        for o in range(8):
                            for u in range(2):
                                nc.tensor.matmul(pss[u], wt[:, o],
                                                 xtts[u][:, o],
                                                 start=(o == 0), stop=(o == 7))
                        for u in range(2):
                            nc.scalar.activation(
                                dst[:, ts(2 * g + u, 512)], pss[u],
                                mybir.ActivationFunctionType.Identity,
                                bias=bias_t[:, bidx:bidx + 1])
                    # rope on first 64 local channels of Q and K
                    for u in range(2):
                        sl = ts(2 * g + u, 512)
                        for t in (Qt, Kt):
                            psr = psRT.tile([P, 512], F32, tag="rt")
                            nc.tensor.matmul(psr, rmat_t,
                                             t[:, sl],
                                             start=True, stop=True)
                            tmp = ropet.tile([HD, 512], F32, tag="tmp")
                            nc.vector.tensor_tensor(
                                tmp, psr[0:HD], sin_t[:, sl],
                                mybir.AluOpType.mult)
                            nc.vector.tensor_tensor(
                                t[0:HD, sl], t[0:HD, sl],
                                cos_t[:, sl], mybir.AluOpType.mult)
                            nc.vector.tensor_tensor(
                                t[0:HD, sl], t[0:HD, sl], tmp,
                                mybir.AluOpType.add)
                        nc.vector.tensor_copy(KzA[0:HD, sl], Kt[0:HD, sl])
                        nc.vector.tensor_copy(KzB[HD:P, sl], Kt[HD:P, sl])
                        # V transpose into per-head layout (+ones col)
                        for s in range(4):
                            kc = (2 * g + u) * 4 + s
                            pst = psRT.tile([P, 512], F32, tag="rt")
                            nc.tensor.transpose(pst[:, 0:P], Vt[:, ts(kc, P)],
                                                ident)
                            nc.vector.tensor_copy(VaugA[:, kc, 0:HD],
                                                  pst[:, 0:HD])
                            nc.vector.tensor_copy(VaugB[:, kc, 0:HD],
                                                  pst[:, HD:P])

            # ---------------- Phase B: attention + output projection
            # One local head per block, S double-buffered so the exp stream
            # on ScalarE (the bottleneck) never waits on S^T latency.  The
            # output projection of a finished (b, nqb) token range runs in a
            # dedicated psum pool and is INJECTED into the middle of the
            # following blocks, well after its normalize has completed, so
            # it never stalls the PE FIFO.
            with tc.tile_pool(name="ptp", bufs=12) as ptp, \
                 tc.tile_pool(name="osb", bufs=3) as osb, \
                 tc.tile_pool(name="nrm", bufs=2) as nrm, \
                 tc.tile_pool(name="spS", bufs=2, space="PSUM") as spS, \
                 tc.tile_pool(name="spO", bufs=1, space="PSUM") as spO, \
                 tc.tile_pool(name="spP", bufs=1, space="PSUM") as spP, \
                 tc.tile_pool(name="spD", bufs=1, space="PSUM") as spD:

                # Keep-warm scratch: the PE clock-gate (HAM) re-throttles to
                # 1.2 GHz after any idle window, and block boundaries leave
                # short PE gaps that put the whole attention phase in a
                # cold-clock equilibrium.  Filler matmuls into this scratch
                # bank bridge those gaps so the PE stays at 2.4 GHz.
                dmy = spD.tile([P, 512], F32, tag="dummy")

                def keep_warm(n=1):
                    for _ in range(n):
                        nc.tensor.matmul(dmy, wos[:, 0:P], wos[:, 0:512],
                                         start=True, stop=True,
                                         skip_group_check=True)

                def oproj_tile(q0, tch):
                    # output projection of one 128-token chunk (both heads);
                    # the two halves use different psum banks (spP and the
                    # keep-warm scratch) so the second matmul never queues
                    # behind the first half's PSUM->SBUF copy
                    t0 = q0 + tch * P
                    Otp = OtT[q0 // 1024]
                    lhs = Otp[:, ts(tch, P)]
                    ost = osb.tile([P, HID], F32, tag="ost")
                    Pps = spP.tile([P, 512], F32, tag="oproj")
                    nc.tensor.matmul(Pps, lhs, wos[:, 0:512],
                                     start=True, stop=True)
                    nc.any.tensor_copy(ost[:, 0:512], Pps)
                    nc.tensor.matmul(dmy, lhs, wos[:, 512:1024],
                                     start=True, stop=True,
                                     skip_group_check=True)
                    nc.any.tensor_copy(ost[:, 512:1024], dmy)
                    nc.sync.dma_start(out[t0:t0 + P, :], ost)

                def normalize(hlo, q0, Ops):
                    # copy out of PSUM right away so the O bank frees up,
                    # then rows 0..63 / row 64 from SBUF.  Processed in nq
                    # halves: shorter reciprocals keep the DVE FIFO
                    # fine-grained so interleaved oproj copies never sit
                    # behind a 6.5us instruction.
                    osum = nrm.tile([HD + 1, 1024], F32, tag="osum")
                    nc.vector.tensor_copy(osum, Ops[0:HD + 1, :])
                    for qt in range(4):
                        rc = nrm.tile([1, 1024], F32, tag="rc",
                                      name="rc")[:, 0:256]
                        nc.vector.reciprocal(
                            rc, osum[HD:HD + 1, ts(qt, 256)])
                        rcb = nrm.tile([HD, 1024], F32, tag="rcb",
                                       name="rcb")[:, 0:256]
                        nc.gpsimd.partition_broadcast(rcb, rc)
                        nc.vector.tensor_tensor(
                            OtT[q0 // 1024][hlo:hlo + HD, ts(qt, 256)],
                            osum[0:HD, ts(qt, 256)],
                            rcb,
                            mybir.AluOpType.mult)

                # (q0, tch) work items for output projection, produced as
                # blocks complete, consumed at injection points
                oproj_queue = []
                blocks = [(b, nqb, h)
                          for b in range(2) for nqb in range(2)
                          for h in range(2)]
                pend = []       # (pv_fn, chunk_idx, Pt) pipeline carry-over
                prev_ctx = None  # (hlo, q0, Ops, bi) awaiting normalize
                for bi, (b, nqb, h) in enumerate(blocks):
                    q0 = b * NB + nqb * 1024
                    hlo = h * HD
                    Vaug = VaugA if h == 0 else VaugB
                    Kz = KzA if h == 0 else KzB

                    def s_exp(i, b=b, q0=q0, Kz=Kz):
                        k0 = b * NB + i * P
                        Sps = spS.tile([P, 1024], F32, tag="S")
                        for hf in range(2):
                            nc.tensor.matmul(
                                Sps[:, ts(hf, 512)],
                                Kz[:, k0:k0 + P],
                                Qt[:, ds(q0 + hf * 512, 512)],
                                start=True, stop=True)
                        Pt = ptp.tile([P, 1024], BF16, tag="P")
                        nc.scalar.activation(
                            Pt, Sps, mybir.ActivationFunctionType.Exp)
                        return Pt

                    # the pipeline carries ACROSS block boundaries: issue
                    # this block's first DEPTH S^T/exp chunks interleaved
                    # with the previous block's tail PVs, then its
                    # normalize, so neither the PE nor ScalarE ever drains
                    # between blocks and the first PV of this block (which
                    # waits on the O-slot handoff) sits deep in the FIFO.
                    DEPTH = 6
                    first_pts = []
                    for k in range(DEPTH):
                        first_pts.append(s_exp(k))
                        if pend:
                            f, idx, pt = pend.pop(0)
                            f(idx, pt)
                    if prev_ctx is not None:
                        phlo, pq0, pOps, pbi = prev_ctx
                        normalize(phlo, pq0, pOps)
                        if pbi % 2 == 1:
                            for tch in range(8):
                                oproj_queue.append((pq0, tch, pbi))
                    keep_warm(2)

                    Ops = spO.tile([P, 1024], F32, tag="O")

                    def pv(i, Pt, Vaug=Vaug, b=b, Ops=Ops):
                        kc = b * 16 + i
                        for hf in range(2):
                            nc.tensor.matmul(
                                Ops[:, ts(hf, 512)],
                                Vaug[:, kc, :],
                                Pt[:, ts(hf, 512)],
                                start=(i == 0), stop=(i == 15),
                                skip_group_check=True)

                    pend = [(pv, k, first_pts[k]) for k in range(DEPTH)]
                    for i in range(DEPTH, 16):
                        pend.append((pv, i, s_exp(i)))
                        f, idx, pt = pend.pop(0)
                        f(idx, pt)
                        if i % 2 == 0 and oproj_queue:
                            src = oproj_queue[0]
                            if bi - src[2] >= 2:
                                oproj_queue.pop(0)
                                oproj_tile(src[0], src[1])
                    prev_ctx = (hlo, q0, Ops, bi)
                # drain the last block's pipeline + normalize
                for f, idx, pt in pend:
                    f(idx, pt)
                phlo, pq0, pOps, pbi = prev_ctx
                # final normalize split by nq halves so the closing drain
                # can start as soon as the first half's scale is ready
                fosum = nrm.tile([HD + 1, 1024], F32, tag="osum")
                nc.vector.tensor_copy(fosum, pOps[0:HD + 1, :])
                for half in range(2):
                    frc = nrm.tile([1, 1024], F32, tag="rc",
                                   name="frc")[:, 0:512]
                    nc.vector.reciprocal(
                        frc, fosum[HD:HD + 1, ts(half, 512)])
                    frcb = nrm.tile([HD, 1024], F32, tag="rcb",
                                    name="frcb")[:, 0:512]
                    nc.gpsimd.partition_broadcast(frcb, frc)
                    nc.vector.tensor_tensor(
                        OtT[pq0 // 1024][phlo:phlo + HD, ts(half, 512)],
                        fosum[0:HD, ts(half, 512)],
                        frcb,
                        mybir.AluOpType.mult)
                for tch in range(8):
                    oproj_queue.append((pq0, tch, pbi))
                # drain remaining output-projection work, alternating
                # between the oproj bank and the keep-warm scratch bank so
                # the two chains run in parallel
                for di, (q0_, tch_, _) in enumerate(oproj_queue):
                    t0 = q0_ + tch_ * P
                    lhs = OtT[q0_ // 1024][:, ts(tch_, P)]
                    ost = osb.tile([P, HID], F32, tag="ost")
                    for hf in range(2):
                        if di % 2 == 0:
                            Pps = spP.tile([P, 512], F32, tag="oproj")
                            nc.tensor.matmul(
                                Pps, lhs, wos[:, ts(hf, 512)],
                                start=True, stop=True)
                            nc.any.tensor_copy(ost[:, ts(hf, 512)], Pps)
                        else:
                            nc.tensor.matmul(
                                dmy, lhs, wos[:, ts(hf, 512)],
                                start=True, stop=True,
                                skip_group_check=True)
                            nc.any.tensor_copy(ost[:, ts(hf, 512)], dmy)
                    nc.sync.dma_start(out[t0:t0 + P, :], ost)
                # keep the scratch tile alive past DCE
                sink = nrm.tile([P, 1], F32, tag="sink")
                nc.vector.tensor_copy(sink, dmy[:, 0:1])

    nc.compile()
    return nc


def _get_nc():
    global _NC_CACHE
    if _NC_CACHE is None:
        _NC_CACHE = build_nc()
    return _NC_CACHE


def shard_inputs(x, rope_cos, rope_sin, Wq, bq, Wk, bk, Wv, bv, Wo, bo):
    """Build per-core input maps."""
    xt = np.ascontiguousarray(x.reshape(NT, HID).T).astype(np.float32)
    cosT = np.ascontiguousarray(rope_cos.reshape(NT, HD).T).astype(np.float32)
    sinT = np.ascontiguousarray(rope_sin.reshape(NT, HD).T).astype(np.float32)
    cos_id = np.ones((HD, NT), np.float32)
    sin_id = np.zeros((HD, NT), np.float32)
    # rotate_half as matrix R: out = R @ t, R[2i,2i+1]=-1, R[2i+1,2i]=+1.
    # matmul computes lhsT.T @ rhs, so pass R.T.
    R = np.zeros((P, P), np.float32)
    idx = np.arange(0, HD, 2)
    R[idx, idx + 1] = -1.0
    R[idx + 1, idx] = 1.0
    rmat = np.ascontiguousarray(R.T)

    in_maps = []
    for c in range(N_CORES):
        lo, hi = c * P, (c + 1) * P
        in_maps.append({
            "xt": xt,
            "wq": np.ascontiguousarray(Wq[:, lo:hi]).astype(np.float32),
            "wk": np.ascontiguousarray(Wk[:, lo:hi]).astype(np.float32),
            "wv": np.ascontiguousarray(Wv[:, lo:hi]).astype(np.float32),
            "wo": np.ascontiguousarray(Wo[lo:hi, :]).astype(ml_dtypes.bfloat16),
            "bias": np.ascontiguousarray(
                np.stack([bq[lo:hi], bk[lo:hi], bv[lo:hi]], axis=1)
            ).astype(np.float32),
            "cos": (cosT if c == 0 else cos_id).astype(ml_dtypes.bfloat16),
            "sin": (sinT if c == 0 else sin_id).astype(ml_dtypes.bfloat16),
            "rmat": rmat,
        })
    return in_maps


def run_device(inputs, trace=False, **kw):
    nc = _get_nc()
    in_maps = shard_inputs(**inputs)
    res = run_bass_kernel_spmd(nc, in_maps, core_ids=list(range(N_CORES)),
                               trace=trace, **kw)
    return res


def gather(res, bo):
    acc = res.results[0]["out"].astype(np.float32).copy()
    for c in range(1, N_CORES):
        acc += res.results[c]["out"]
    acc += bo[None, :].astype(np.float32)
    return acc.reshape(2, NB, HID)


def kernel(**inputs):
    # NRT_EXEC_UNIT_UNRECOVERABLE crashes are transient on this fleet;
    # one retry rescues the run.
    try:
        res = run_device(inputs, trace=False)
    except Exception:
        res = run_device(inputs, trace=False)
    return gather(res, np.asarray(inputs["bo"], np.float32))
